# revision 62
# baseline (speedup 1.0000x reference)
"""Bass/Trainium2 kernel for a 16-head causal MHA block with partial rotary.

Problem shapes (hardcoded): x [2,2048,1024] fp32, Wq/Wk/Wv/Wo [1024,1024],
mask = causal tril [2048,2048] (hardcoded causality; mask input unused).

Sharding over 8 NeuronCores: core c handles batch c//4 and the 4 heads
h0 = (c%4)*4 .. h0+3 (tensor parallel on heads).  Each core computes its
partial output y_h @ Wo[h-block] summed over its 4 heads; the host adds the
4 per-batch partials.

Design notes:
- Host sends x already transposed (xT [1024, 2048]) and bf16: kills the
  16 PE transposes + 16 big DVE evictions an in-kernel transpose costs.
- All matmul operands bf16; PSUM accumulation fp32.
- QK runs per HEAD PAIR: the two heads of a pt-tile sit at row offsets
  0/64, so their K=64 matmuls target disjoint PE row-groups
  (tile_position row tiling) and execute CONCURRENTLY on the 128x128
  array (measured ~30us on HW).  Both heads' logits for one j-tile share
  a [128, 2, 512] PSUM tile and one exp instruction covers both.
  (Do NOT try the same trick for AV by splitting its K=128 contraction:
  two concurrent matmuls accumulating into the same PSUM bank crash the
  device, and matmul outputs at base partition 64 fail neuronx compile.)
- Causal wedge at 128-column granularity; only [128,128] diagonal blocks
  get an affine_select mask (Pool), one op per head pair.
- Softmax denominators ride as a 65th ones-column in the AV matmul; 1/s is
  broadcast across partitions with a K=1 outer-product matmul (f32r).
- All input DMAs are folded (rearranged APs, e.g. dram (d p) c -> p d c)
  into one descriptor per tensor / x-group: 29 dma_starts total.
- Fully software-pipelined by groups of 4 seq-tiles: projections for group
  g+1 interleave between attention blocks of group g.
"""

import numpy as np

S, D, H, HD, PROT = 2048, 1024, 16, 64, 32
NHC = 4            # heads per core
SEQT = S // 128    # 16
DCH = D // 128     # 8
NIC = 4            # i-chunks of 512

_CACHED = {}


def _rot_tables():
    invf = 10000.0 ** (-np.arange(0, PROT, 2, dtype=np.float64) / PROT)  # [16]
    ang = np.arange(S, dtype=np.float64)[None, :] * invf[:, None]        # [16, S]
    C64 = np.ones((64, S), np.float64)
    S64 = np.zeros((64, S), np.float64)
    for d in range(PROT):
        C64[d] = np.cos(ang[d // 2])
        S64[d] = (1.0 if d % 2 else -1.0) * np.sin(ang[d // 2])
    CT = np.concatenate([C64, C64], 0).astype(np.float32)
    ST = np.concatenate([S64, S64], 0).astype(np.float32)
    return CT, ST


def build_nc(reps=1, ablate=(), dbg=False):
    import concourse.bacc as bacc
    import concourse.mybir as mybir
    from concourse.tile import TileContext

    F32 = mybir.dt.float32
    BF16 = mybir.dt.bfloat16
    AF = mybir.ActivationFunctionType
    ALU = mybir.AluOpType

    nc = bacc.Bacc("TRN2", target_bir_lowering=False, debug=False)

    xt_d = nc.dram_tensor("xt", [D, S], BF16, kind="ExternalInput").ap()
    wq_d = nc.dram_tensor("wq", [D, 256], BF16, kind="ExternalInput").ap()
    wk_d = nc.dram_tensor("wk", [D, 256], BF16, kind="ExternalInput").ap()
    wv_d = nc.dram_tensor("wv", [D, 256], BF16, kind="ExternalInput").ap()
    wo_d = nc.dram_tensor("wo", [256, D], BF16, kind="ExternalInput").ap()
    out_d = nc.dram_tensor("out", [S, D], BF16, kind="ExternalOutput").ap()

    import ml_dtypes
    CT, ST = _rot_tables()
    ct_d = nc.inline_tensor(CT.astype(ml_dtypes.bfloat16), "ct_const").ap()
    st_d = nc.inline_tensor(ST.astype(ml_dtypes.bfloat16), "st_const").ap()
    ones64_d = nc.inline_tensor(np.ones((1, 64), np.float32), "ones64_const").ap()

    SWAP_MASK = [i ^ 1 for i in range(32)]

    with TileContext(nc) as tc:
      for _rep in range(reps):
        with (
            tc.tile_pool(name="persist", bufs=1) as pp,
            tc.tile_pool(name="small", bufs=2) as sp,
        ):
            qT = [pp.tile([128, S], BF16, tag=f"qT{i}", name=f"qT{i}") for i in range(2)]
            kT = [pp.tile([128, S], BF16, tag=f"kT{i}", name=f"kT{i}") for i in range(2)]
            vt = [pp.tile([128, NHC * 65], BF16, tag=f"vt{i}", name=f"vt{i}") for i in range(SEQT)]
            wo_sb = [pp.tile([128, D], BF16, tag=f"wo{i}", name=f"wo{i}") for i in range(2)]
            yT = [pp.tile([128, S], BF16, tag=f"yT{i}", name=f"yT{i}") for i in range(2)]
            xT = pp.tile([128, DCH * S], BF16, tag="xT")  # chunk d at cols [d*S,(d+1)*S)
            xT3 = xT[:].rearrange("p (d s) -> p d s", d=DCH, s=S)
            F32R = mybir.dt.float32r
            ones64 = pp.tile([1, 64], F32R, tag="ones64")
            nc.scalar.dma_start(out=ones64[:], in_=ones64_d[:].bitcast(F32R))

            ct_sb = pp.tile([128, S], BF16, tag="ct", name="ct")
            st_sb = pp.tile([128, S], BF16, tag="st", name="st")
            # all 8 d-chunks of each weight in one tile: chunk d at cols
            # [d*256, (d+1)*256)
            w_sb = {
                t: pp.tile([128, DCH * 256], BF16, tag=f"w{t}", name=f"w{t}")
                for t in ("q", "k", "v")
            }

            def load_w(name, w_d):
                # one folded DMA: dram row d*128+p -> partition p, chunk d
                src = w_d[:].rearrange("(d p) c -> p d c", d=DCH, p=128)
                dst = w_sb[name][:].rearrange("p (d c) -> p d c", d=DCH, c=256)
                nc.scalar.dma_start(out=dst, in_=src)

            with (
                tc.tile_pool(name="epool", bufs=34) as ep,
                tc.tile_pool(name="rot", bufs=4) as rp,
                tc.tile_pool(name="opool", bufs=4) as op,
                tc.tile_pool(name="psB", bufs=2, space="PSUM") as psB,
                tc.tile_pool(name="psL", bufs=2, space="PSUM") as psL,
                tc.tile_pool(name="psY", bufs=2, space="PSUM") as psY,
            ):
                def emit_xload(g, split_first=False):
                    # xT arrives pre-transposed from the host; one folded DMA
                    # per seq-group covers all 8 d-chunks
                    c0 = 4 * g * 128
                    src = xt_d[:].rearrange("(d p) s -> p d s", d=DCH, p=128)
                    if split_first:
                        # prologue: land the first seq-tile fast so vproj(0)
                        # can start while the rest streams
                        nc.sync.dma_start(
                            out=xT3[:, :, c0:c0 + 128],
                            in_=src[:, :, c0:c0 + 128],
                        )
                        nc.sync.dma_start(
                            out=xT3[:, :, c0 + 128:c0 + 512],
                            in_=src[:, :, c0 + 128:c0 + 512],
                        )
                    else:
                        nc.sync.dma_start(
                            out=xT3[:, :, c0:c0 + 512],
                            in_=src[:, :, c0:c0 + 512],
                        )

                def emit_vones(st):
                    vt_ones = vt[st][:].rearrange("p (h c) -> p h c", h=NHC, c=65)[:, :, 64:65]
                    nc.gpsimd.memset(vt_ones, 1.0)

                def emit_vproj(st):
                    ps = psB.tile([128, 256], F32, tag="proj")
                    for d in range(DCH):
                        nc.tensor.matmul(
                            ps[:],
                            xT[:, d * S + st * 128: d * S + st * 128 + 128],
                            w_sb["v"][:, d * 256:(d + 1) * 256],
                            start=(d == 0), stop=(d == DCH - 1),
                        )
                    dst = vt[st][:].rearrange("p (h c) -> p h c", h=NHC, c=65)[:, :, :64]
                    src = ps[:].rearrange("p (h c) -> p h c", h=NHC, c=64)
                    nc.vector.tensor_copy(dst, src)

                def emit_qkproj(name, pt, sc, dstT):
                    ps = psB.tile([128, 512], F32, tag="proj")
                    for d in range(DCH):
                        nc.tensor.matmul(
                            ps[:],
                            w_sb[name][:, d * 256 + pt * 128: d * 256 + pt * 128 + 128],
                            xT[:, d * S + sc * 512: d * S + sc * 512 + 512],
                            start=(d == 0), stop=(d == DCH - 1),
                        )
                    dst = dstT[pt][:, sc * 512:(sc + 1) * 512]
                    if "rotary" in ablate:
                        nc.scalar.copy(out=dst, in_=ps[:])
                    else:
                        # dst = t0*C + pairswap(t0)*S  (C=1,S=0 on non-rotary
                        # rows).  One ACT copy evicts PSUM to bf16; DVE/Pool
                        # then run at 2x 16-bit rate from SBUF.
                        t0 = rp.tile([128, 512], BF16, tag="t0", name="t0")
                        sw = rp.tile([128, 512], BF16, tag="sw", name="sw")
                        nc.scalar.copy(out=t0[:], in_=ps[:])
                        nc.vector.stream_shuffle(sw[:], t0[:], SWAP_MASK)
                        nc.vector.tensor_mul(
                            dst, t0[:], ct_sb[:, sc * 512:(sc + 1) * 512])
                        nc.gpsimd.tensor_mul(
                            sw[:], sw[:], st_sb[:, sc * 512:(sc + 1) * 512])
                        nc.gpsimd.tensor_add(dst, dst, sw[:])

                def emit_qk_pair(ic, hp):
                    """QK + exp + causal mask for head pair (2hp, 2hp+1) on
                    one i-chunk.  The two heads sit at row offsets 0/64 of the
                    same pt tile, so their matmuls target disjoint PE
                    row-groups (tile_position) and run concurrently.  Both
                    heads' logits for one j-tile share a [128, 2, 512] PSUM
                    tile; one exp covers both."""
                    i0 = ic * 512
                    njt = 4 * ic + 4
                    pt = hp
                    yt_ps = [psY.tile([65, 512], F32, tag="yt", name="yt")
                             for _ in range(2)]
                    stash = [None, None]
                    es = []
                    for jt in range(njt):
                        w = jt * 128 - i0
                        lo = max(0, w)
                        l_ps = psL.tile([128, 2 * 512], F32, tag="l", name="l")
                        l3 = l_ps[:].rearrange("p (h c) -> p h c", h=2, c=512)
                        e = ep.tile([128, 2 * 512], BF16, tag="e", name="e")
                        e3 = e[:].rearrange("p (h c) -> p h c", h=2, c=512)
                        if "qk" not in ablate:
                            for hh in range(2):
                                r0 = hh * 64
                                nc.tensor.matmul(
                                    l3[:, hh, lo:512],
                                    kT[pt][r0:r0 + 64, jt * 128:(jt + 1) * 128],
                                    qT[pt][r0:r0 + 64, i0 + lo:i0 + 512],
                                    start=True, stop=True,
                                    tile_position=(r0, 0),
                                )
                        if "exp" in ablate:
                            nc.vector.tensor_copy(e3[:, :, lo:512], l3[:, :, lo:512])
                        elif "smallexp" in ablate:
                            # timing probe: keep the dep chain, cut ACT work
                            nc.scalar.activation(e3[:, :, lo:lo + 64],
                                                 l3[:, :, lo:lo + 64],
                                                 AF.Exp, scale=0.125)
                        else:
                            nc.scalar.activation(e3[:, :, lo:512], l3[:, :, lo:512],
                                                 AF.Exp, scale=0.125)
                        if w >= 0 and "affine" not in ablate:
                            # partial 128-col diagonal block: keep u >= p,
                            # both heads in one op
                            nc.gpsimd.affine_select(
                                out=e3[:, :, lo:lo + 128],
                                in_=e3[:, :, lo:lo + 128],
                                compare_op=ALU.is_ge, fill=0.0,
                                base=0, channel_multiplier=-1,
                                pattern=[[0, 2], [1, 128]],
                            )
                        es.append((jt, lo, e3))
                    return (ic, hp, yt_ps, es, stash)

                def emit_av_head(state, hh):
                    """AV matmuls + 1/s + eviction for one head of a qk-pair
                    state.  The paired normalize (emit_norm_pair) runs after
                    both heads, so the two bc broadcasts go back-to-back on
                    disjoint PE col-groups."""
                    ic, hp, yt_ps, es, stash = state
                    njt = 4 * ic + 4
                    h = 2 * hp + hh
                    ytp = yt_ps[hh]
                    for jt, lo, e3 in es:
                        if "av" not in ablate:
                            nc.tensor.matmul(
                                ytp[:, lo:512],
                                vt[jt][:, h * 65: h * 65 + 65],
                                e3[:, hh, lo:512],
                                start=(jt == 0),
                                stop=(jt == njt - 1),
                            )
                    if "norm" not in ablate:
                        rs = sp.tile([1, 512], F32R, tag="rs", name="rs")
                        with nc.allow_low_precision(reason="1/s via f32r is fine"):
                            nc.vector.reciprocal(rs[0:1, :], ytp[64:65, :])
                        # evict yt to SBUF (one PSUM read port)
                        yu = sp.tile([64, 512], BF16, tag="yu", name="yu")
                        nc.vector.tensor_copy(yu[:], ytp[0:64, :])
                        stash[hh] = (rs, yu)
                    if hh == 1 and "norm" not in ablate:
                        emit_norm_pair(state)

                def emit_norm_pair(state):
                    """Broadcast both heads' 1/s with K=1 outer products into
                    one PSUM tile, then scale."""
                    ic, hp, yt_ps, es, stash = state
                    i0 = ic * 512
                    pt = hp
                    bcs = []
                    for hh in range(2):
                        rs, yu = stash[hh]
                        bc = psY.tile([64, 512], F32, tag="yt", name="bc")
                        nc.tensor.matmul(
                            bc[:], ones64[0:1, :], rs[0:1, :],
                            start=True, stop=True,
                        )
                        bcs.append(bc)
                    for hh in range(2):
                        rs, yu = stash[hh]
                        r0 = hh * 64
                        nc.vector.tensor_mul(
                            yT[pt][r0:r0 + 64, i0:i0 + 512], yu[:],
                            bcs[hh][:]
                        )

                def emit_oproj(ic):
                    # mid-rep oprojs share the psB ring with projections; the
                    # FINAL group's oproj instead rides psY (idle once the
                    # last attention block is normalized), so the next rep's
                    # first vproj never waits on this rep's last oproj
                    # eviction at the seam
                    pool, tag = (psY, "yt") if ic == NIC - 1 else (psB, "proj")
                    for st in range(4 * ic, 4 * ic + 4):
                        # dc inner so each yT stationary is reused for 2 matmuls
                        pss = [pool.tile([128, 512], F32, tag=tag, name="o")
                               for _ in range(2)]
                        for pt in range(2):
                            for dc in range(2):
                                nc.tensor.matmul(
                                    pss[dc][:],
                                    yT[pt][:, st * 128:(st + 1) * 128],
                                    wo_sb[pt][:, dc * 512:(dc + 1) * 512],
                                    start=(pt == 0), stop=(pt == 1),
                                )
                        ob = op.tile([128, 1024], BF16, tag="ob", name="ob")
                        if "noob" in ablate:
                            nc.vector.tensor_copy(ob[:, 0:64], pss[0][:, 0:64])
                        else:
                            for dc in range(2):
                                nc.vector.tensor_copy(
                                    ob[:, dc * 512:(dc + 1) * 512], pss[dc][:])
                        # gpsimd SWDGE: inputs keep both HWDGE queues
                        # (x on SP, weights on ACT), so the next rep's loads
                        # never queue behind this rep's output stores
                        nc.gpsimd.dma_start(
                            out=out_d[st * 128:(st + 1) * 128, :],
                            in_=ob[:],
                        )

                def proj_work(g):
                    """PE work pieces that prepare group g (projections), to be
                    interleaved between attention blocks of group g-1.  vproj
                    (DVE eviction) and qkproj (ACT eviction) alternate so the
                    psB ring drains on both engines in parallel."""
                    ww = [lambda g=g: emit_xload(g)]
                    for st in range(4 * g, 4 * g + 4):
                        ww.append(lambda st=st: emit_vproj(st))
                    for pt in range(2):
                        ww.append(lambda pt=pt, g=g: emit_qkproj("k", pt, g, kT))
                        ww.append(lambda pt=pt, g=g: emit_qkproj("q", pt, g, qT))
                    return ww

                # group 0 prologue: x chunk + weights first so vproj starts
                # ASAP, then remaining consts in need order
                work0 = proj_work(0)
                work0.pop(0)
                emit_xload(0, split_first=True)
                for st in range(SEQT):
                    emit_vones(st)
                load_w("v", wv_d)
                load_w("k", wk_d)
                nc.scalar.dma_start(out=ct_sb[:], in_=ct_d[:])
                nc.scalar.dma_start(out=st_sb[:], in_=st_d[:])
                load_w("q", wq_d)
                for i in range(2):
                    nc.scalar.dma_start(out=wo_sb[i][:], in_=wo_d[i * 128:(i + 1) * 128, :])
                for w in work0:
                    w()

                pending = []
                for g in range(NIC):
                    work = proj_work(g + 1) if g + 1 < NIC else []
                    if g > 0:
                        work.append(lambda g=g: emit_oproj(g - 1))
                    npieces = len(work)
                    for hp in range(2):
                        st = emit_qk_pair(g, hp)
                        pending.append((st, 0))
                        pending.append((st, 1))
                        # spread proj/oproj pieces between attention blocks
                        take = (npieces * (hp + 1)) // 2 - (npieces * hp) // 2
                        nt1 = take // 2
                        for _ in range(nt1):
                            work.pop(0)()
                        if len(pending) > 2:
                            emit_av_head(*pending.pop(0))
                        for _ in range(take - nt1):
                            work.pop(0)()
                        if len(pending) > 2:
                            emit_av_head(*pending.pop(0))
                    while pending:
                        emit_av_head(*pending.pop(0))
                emit_oproj(NIC - 1)
                if dbg:
                    for nm, tiles in (("qT", qT), ("kT", kT), ("yT", yT)):
                        for i, t in enumerate(tiles):
                            dd = nc.dram_tensor(f"dbg_{nm}{i}", [128, S], BF16,
                                                kind="ExternalOutput").ap()
                            nc.sync.dma_start(out=dd[:], in_=t[:])
                    for i in (0, 7, 15):
                        dd = nc.dram_tensor(f"dbg_vt{i}", [128, NHC * 65], BF16,
                                            kind="ExternalOutput").ap()
                        nc.sync.dma_start(out=dd[:], in_=vt[i][:])

    nc.compile()
    return nc


def _in_maps(x, Wq, Wk, Wv, Wo):
    import ml_dtypes
    bf = ml_dtypes.bfloat16
    maps = []
    xts = [np.ascontiguousarray(x[b].T).astype(bf) for b in range(2)]
    for core in range(8):
        b, hg = core // 4, core % 4
        c0 = hg * 4 * HD
        maps.append({
            "xt": xts[b],
            "wq": np.ascontiguousarray(Wq[:, c0:c0 + 256]).astype(bf),
            "wk": np.ascontiguousarray(Wk[:, c0:c0 + 256]).astype(bf),
            "wv": np.ascontiguousarray(Wv[:, c0:c0 + 256]).astype(bf),
            "wo": np.ascontiguousarray(Wo[c0:c0 + 256, :]).astype(bf),
        })
    return maps


def kernel(x, mask, Wq, Wk, Wv, Wo):
    from concourse.bass_utils import run_bass_kernel_spmd

    x, Wq, Wk, Wv, Wo = (np.asarray(a, np.float32) for a in (x, Wq, Wk, Wv, Wo))
    if "nc" not in _CACHED:
        _CACHED["nc"] = build_nc()
    res = run_bass_kernel_spmd(_CACHED["nc"], _in_maps(x, Wq, Wk, Wv, Wo),
                               core_ids=list(range(8)))
    out = np.zeros((2, S, D), np.float32)
    for core in range(8):
        out[core // 4] += res.results[core]["out"].astype(np.float32)
    return out
